# revision 13
# baseline (speedup 1.0000x reference)
"""Trainium2 Bass kernel for DPNET (gnn_message_passing), 8-core SPMD.

Sharding: node dim N=4096 split into 8 row-blocks of 512. Each core owns the
same 512 rows for all 3 views: they serve as its block of the attention
contraction dim (j) and, after a ReduceScatter, as its output rows.

Key tricks:
  - exp(leaky(z))-masked softmax weights WITHOUT any big ACT pass:
    exp(leaky(e1_i+e2_j)) = u4_i * max(r_i*w1_j, w4_j) with r=exp(.75 e1),
    u4=exp(.25 e1), w1=exp(e2), w4=exp(.25 e2). The per-i factor u4_i scales
    numerator AND denominator of the row softmax -> cancels, so only
    W'[j,i] = adj * max(r_i*w1_j, w4_j) is needed. r is AllGathered (fp16,
    one fused collective for all 3 views); w1/w4 are per-own-row scalars.
  - adjacency mask applied via SWDGE accumulate-ADD DMA (the only CCE op
    walrus accepts): adj stored fp8e5 as {-32768, 0}, cast+added onto Wm
    during the DMA, then one relu tensor_scalar (4x-mode eligible) replaces
    the 2x-capped tensor_tensor mult. adj HBM bytes halve (1B/elem).
  - warmup collective at t=0 absorbs CC-core startup / cross-core skew.
  - softmax denominator via a ones-column in the matmul rhs; attention
    weights/partials fp16 (fp32 PSUM accum); fp16 ReduceScatter.
  - Inner FE attention exp(q_i*k_o/s) (|x|<=0.75) as a degree-DFE Taylor
    series: per-node moments m_d = sum_o k^d v / n_d = sum_o k^d via PE
    ones-reduction column matmuls; assembly with scalar_tensor_tensor.
  - all matmuls fp16 (FWL stays enabled, no fp32 PE mode switches).
  - attention PSUM banks packed 2 i-chunks wide -> half the ACT copies.
"""
import math
import numpy as np
import ml_dtypes

import concourse.bass as bass
import concourse.bacc as bacc
import concourse.mybir as mybir
import concourse.tile as tile
from concourse.bass_utils import run_bass_kernel_spmd

V, N, D, H, C = 3, 4096, 512, 128, 5
NCORES = 8
NB = N // NCORES            # 512 rows per core
JC = NB // 128              # 4 chunks of own rows
IC = N // 128               # 32 i-chunks
DC = D // 128               # 4 contraction chunks for D
DFE = 4                     # FE Taylor degree
NMOM = 2 * DFE + 1          # m_0..m_DFE, n_1..n_DFE
f32 = mybir.dt.float32
fp16 = mybir.dt.float16
fp8e5 = mybir.dt.float8e5
AF = mybir.ActivationFunctionType
OP = mybir.AluOpType
fp16np = np.float16

# fp16 wpack column layout: a1 | a2
PK_A1, PK_A2 = 0, 1
PK_X = 2
# fp16 pack: qws(128) | kw(128) | vw(128) | fcw(128) | confw | mmw(5)
PG_QW, PG_KW, PG_VW, PG_FW = 0, H, 2 * H, 3 * H
PG_CW = 4 * H
PG_MMW = 4 * H + 1
PG_X = 4 * H + 1 + C
# fp16 row pack: qbs(128) | kb(128) | vb(128) | mlpb(128) | fcb(128) | confb | mmb(5)
RP_QB, RP_KB, RP_VB, RP_MB, RP_FB = 0, H, 2 * H, 3 * H, 4 * H
RP_CB = 5 * H
RP_MMB = 5 * H + 1
RP_X = 5 * H + 1 + C

_CACHE = {}
SIM_NO_CC = False  # replace collectives with DMA stubs (for TimelineSim)
ACCUM_ADD = False  # mask via gpsimd cast+add-accum DMA + relu instead of mult
MASK_BIG = 32768.0


def build_nc():
    nc = bacc.Bacc("TRN2", target_bir_lowering=False, num_devices=NCORES)

    adjm_d = nc.dram_tensor("adjm", [V, NB, N], fp8e5, kind="ExternalInput")
    dataT16_d = nc.dram_tensor("dataT16", [V, 128, DC * NB], fp16,
                               kind="ExternalInput")
    gacw16_d = nc.dram_tensor("gacw16", [V, 128, DC * H], fp16,
                              kind="ExternalInput")
    mlpw16_d = nc.dram_tensor("mlpw16", [V, 128, DC * H], fp16,
                              kind="ExternalInput")
    gacb_d = nc.dram_tensor("gacb", [V, H], f32, kind="ExternalInput")
    wpack_d = nc.dram_tensor("wpack", [V, 128, PK_X], fp16, kind="ExternalInput")
    wp16_d = nc.dram_tensor("wp16", [V, 128, PG_X], fp16, kind="ExternalInput")
    rp16_d = nc.dram_tensor("rp16", [V, 1, RP_X], fp16, kind="ExternalInput")
    warm_d = nc.inline_tensor(np.ones((1, 4), fp16np), name="warm")

    out_d = nc.dram_tensor("out", [NB, C], f32, kind="ExternalOutput")

    fct_d = nc.inline_tensor(
        np.array([[1.0 / math.factorial(d) for d in range(DFE + 1)]
                  + [1.0 / math.factorial(d) for d in range(1, DFE + 1)]],
                 np.float32), name="fct")
    ident16_d = nc.inline_tensor(np.eye(128, dtype=fp16np), name="ident16")

    from contextlib import ExitStack
    with tile.TileContext(nc) as tc:
        with ExitStack() as stk:
            ep = lambda *a, **k: stk.enter_context(tc.tile_pool(*a, **k))
            cpool = ep(name="const", bufs=1)
            dpool = ep(name="dat", bufs=3)
            wpool = ep(name="wts", bufs=2)
            vpool = ep(name="persist", bufs=V)
            rhspool = ep(name="rhsp", bufs=V * JC)
            spool = ep(name="scratch", bufs=3)
            e1pool = ep(name="e1bp", bufs=2)
            wmpool = ep(name="wmp", bufs=8)
            apool = ep(name="adjp", bufs=5)
            a4pool = ep(name="att4p", bufs=4)
            fepool = ep(name="fe2", bufs=2)
            fgpool = ep(name="feg", bufs=2)
            fe5pool = ep(name="fe5", bufs=3)
            qpool = ep(name="qp", bufs=4)
            mlppool = ep(name="mlpp", bufs=V * JC)
            psA = ep(name="psA", bufs=2, space="PSUM")
            psB = ep(name="psB", bufs=3, space="PSUM")
            psC = ep(name="psC", bufs=2, space="PSUM")
            psM = ep(name="psM", bufs=1, space="PSUM")
            drpool = ep(name="dram", bufs=1, space="DRAM")

            # ---------- constants ----------
            ident16 = cpool.tile([128, 128], fp16, tag="c1")
            nc.sync.dma_start(ident16[:], ident16_d[:])
            ones16c = cpool.tile([128, 1], fp16, tag="c3")
            nc.vector.memset(ones16c[:], 1.0)
            ones_row16 = cpool.tile([1, 128], fp16, tag="c5")
            nc.vector.memset(ones_row16[:], 1.0)
            fct_bc = cpool.tile([128, NMOM], f32, tag="c7")
            nc.sync.dma_start(fct_bc[:], fct_d[0:1, :].partition_broadcast(128))
            ones_nb16 = cpool.tile([1, NB], fp16, tag="c8")
            nc.vector.memset(ones_nb16[:], 1.0)

            partials, rsouts = [], []
            agi_all = drpool.tile([V, NB], fp16, tag="agi")
            ago_all = drpool.tile([NCORES, V, NB], fp16,
                                  addr_space="Local" if SIM_NO_CC else "Shared",
                                  tag="ago")
            warm_o = drpool.tile([NCORES, 1, 4], fp16,
                                 addr_space="Local" if SIM_NO_CC else "Shared",
                                 tag="warm_o")
            for _pv in range(V):
                pt = [drpool.tile([NCORES, NB // 2, H + 1], fp16,
                                  tag=f"part{_pv}h{_h}", name=f"pt{_pv}{_h}") for _h in range(2)]
                partials.append(pt)
                rt_ = [drpool.tile([NB // 2, H + 1], fp16,
                                   tag=f"rsout{_pv}h{_h}", name=f"rt{_pv}{_h}") for _h in range(2)]
                rsouts.append(rt_)

            # warmup collective: absorbs CC-core startup / cross-core skew
            def warmup_cc():
                if not SIM_NO_CC:
                    nc.gpsimd.collective_compute(
                        "AllGather", OP.bypass,
                        replica_groups=[list(range(NCORES))],
                        ins=[warm_d[:, :]], outs=[warm_o.opt()])

            # ---------- P1 per view: hT, r (+gather), w1/w4, rhs tiles ----------
            rhs_sb = [[None] * JC for _ in range(V)]
            w1_sb = [None] * V
            w4_sb = [None] * V
            dt16_all = [None] * V

            def p1(v):
                dt16 = dpool.tile([128, DC * NB], fp16, tag="dt16")
                nc.sync.dma_start(dt16[:], dataT16_d[v, :, :])
                dt16_all[v] = dt16
                gw16 = wpool.tile([128, DC * H], fp16, tag="gw16")
                nc.scalar.dma_start(gw16[:], gacw16_d[v, :, :])
                wp = wpool.tile([128, PK_X], fp16, tag="wp")
                nc.scalar.dma_start(wp[:], wpack_d[v, :, :])
                # hT = (data @ gac_w).T : lhsT=gw chunk [d,H], rhs=dataT chunk
                hT_ps = psM.tile([128, NB], f32, tag="mm")
                for dc in range(DC):
                    nc.tensor.matmul(
                        hT_ps[:], gw16[:, dc * H:(dc + 1) * H],
                        dt16[:, dc * NB:(dc + 1) * NB],
                        start=(dc == 0), stop=(dc == DC - 1))
                hT = cpool.tile([128, NB], fp16, tag="hT")
                nc.scalar.copy(hT[:], hT_ps[:])
                e1_ps = psB.tile([1, NB], f32, tag="g")
                nc.tensor.matmul(e1_ps[:], wp[:, PK_A1:PK_A1 + 1], hT[:],
                                 start=True, stop=True)
                # r = exp(0.75*e1) (the u4=exp(.25 e1) factor cancels in the
                # softmax normalization, so only r is gathered)
                rrow = cpool.tile([1, NB], fp16, tag="rrow")
                nc.scalar.activation(rrow[:], e1_ps[:], AF.Exp,
                                     bias=0.0, scale=0.75)
                nc.scalar.dma_start(agi_all[v:v + 1, :], rrow[:])
                e2c = vpool.tile([128, JC], f32, tag="e2c")
                for jc in range(JC):
                    e2_ps = psB.tile([128, 1], f32, tag="g")
                    nc.tensor.matmul(
                        e2_ps[:], hT[:, jc * 128:(jc + 1) * 128],
                        wp[:, PK_A2:PK_A2 + 1], start=True, stop=True)
                    nc.scalar.copy(e2c[:, jc:jc + 1], e2_ps[:])
                w1c = vpool.tile([128, JC], f32, tag="w1c")
                nc.scalar.activation(w1c[:], e2c[:], AF.Exp, bias=0.0, scale=1.0)
                w4c = vpool.tile([128, JC], f32, tag="w4c")
                nc.scalar.activation(w4c[:], e2c[:], AF.Exp, bias=0.0, scale=0.25)
                w1_sb[v] = w1c
                w4_sb[v] = w4c
                for jc in range(JC):
                    t_ps = psC.tile([128, 128], fp16, tag="tp")
                    nc.tensor.transpose(
                        t_ps[:], hT[:, jc * 128:(jc + 1) * 128], ident16[:])
                    rt = rhspool.tile([128, H + 1], fp16, tag="rhs")
                    nc.scalar.copy(rt[:, 0:H], t_ps[:])
                    nc.vector.memset(rt[:, H:H + 1], 1.0)
                    rhs_sb[v][jc] = rt

            # single fused AllGather for all three views' r rows
            def ag_all():
                if SIM_NO_CC:
                    nc.sync.dma_start(
                        ago_all[:, :, :],
                        agi_all[:, :].partition_broadcast(NCORES))
                else:
                    nc.gpsimd.collective_compute(
                        "AllGather", OP.bypass,
                        replica_groups=[list(range(NCORES))],
                        ins=[agi_all.opt()], outs=[ago_all.opt()])

            # ---------- P3/P4 per view: attention + partials + RS ----------
            madj_all = {}

            def p3_adj(v):
                # AG-independent: prefetch+cast the adjacency tiles early
                for jc in range(JC):
                    madj = apool.tile([128, N], fp16, tag="madj")
                    nc.gpsimd.dma_start(
                        madj[:], adjm_d[v, jc * 128:(jc + 1) * 128, :])
                    madj_all[(v, jc)] = madj

            def p3(v):
                rb = e1pool.tile([128, N], fp16, tag="rb")
                nc.sync.dma_start(
                    rb[:].rearrange("p (k r) -> p k r", k=NCORES),
                    ago_all[:, v:v + 1, :].rearrange("k o r -> o k r")
                    .partition_broadcast(128))
                wms = []
                for jc in range(JC):
                    Wm = wmpool.tile([128, N], fp16, tag="Wm")
                    nc.vector.tensor_scalar(
                        out=Wm[:], in0=rb[:],
                        scalar1=w1_sb[v][:, jc:jc + 1],
                        scalar2=w4_sb[v][:, jc:jc + 1],
                        op0=OP.mult, op1=OP.max)
                    madj = madj_all.pop((v, jc))
                    nc.vector.tensor_tensor(Wm[:], Wm[:], madj[:], OP.mult)
                    
                    wms.append(Wm)
                # attention: PSUM banks packed 2 i-chunks wide; one fp16 att2
                # copy + one partials DMA per pair. Even pairs (first half of
                # every core's slot) complete first -> half-RS starts early.
                for half in range(2):
                    for ko in range(NCORES):
                        gp = ko * 2 + half
                        att_ps = psA.tile([128, 2 * (H + 1)], f32, tag="att")
                        for c2 in range(2):
                            g = gp * 2 + c2
                            osl = slice(c2 * (H + 1), (c2 + 1) * (H + 1))
                            for jc in range(JC):
                                nc.tensor.matmul(
                                    att_ps[:, osl],
                                    wms[jc][:, g * 128:(g + 1) * 128],
                                    rhs_sb[v][jc][:],
                                    start=(jc == 0), stop=(jc == JC - 1))
                        att2 = a4pool.tile([128, 2 * (H + 1)], fp16, tag="att4")
                        nc.scalar.copy(att2[:], att_ps[:])
                        nc.sync.dma_start(
                            partials[v][half][ko, :, :]
                            .rearrange("(c p) h -> p c h", p=128),
                            att2[:].rearrange("p (c h) -> p c h", c=2))
                    if SIM_NO_CC:
                        nc.sync.dma_start(rsouts[v][half][:, :],
                                          partials[v][half][0, :, :])
                    else:
                        nc.gpsimd.collective_compute(
                            "ReduceScatter", OP.add,
                            replica_groups=[list(range(NCORES))],
                            ins=[partials[v][half].opt()],
                            outs=[rsouts[v][half].opt()])

            # ---------- P5..P9 per view: fully per-chunk pipelined ----------
            mm_ps = psM.tile([C, NB], f32, tag="mm")

            mlpn_all = [None] * V

            def p5_mlp(v):
                # mlp branch: independent of the ReduceScatter -> compute
                # before the collectives to fill the startup dead zone
                mw16 = wpool.tile([128, DC * H], fp16, tag="mw16")
                nc.scalar.dma_start(mw16[:], mlpw16_d[v, :, :])
                rpm = wpool.tile([1, RP_X], fp16, tag="rpm")
                nc.scalar.dma_start(rpm[:], rp16_d[v, :, :])
                dt16 = dt16_all[v]
                mlpn = []
                for jc in range(JC):
                    mlp_ps = psB.tile([128, H], f32, tag="g")
                    for dc in range(DC):
                        nc.tensor.matmul(
                            mlp_ps[:],
                            dt16[:, dc * NB + jc * 128:dc * NB + (jc + 1) * 128],
                            mw16[:, dc * H:(dc + 1) * H],
                            start=(dc == 0), stop=False)
                    nc.tensor.matmul(mlp_ps[:], ones_row16[:],
                                     rpm[:, RP_MB:RP_MB + H],
                                     start=False, stop=True)
                    mn = mlppool.tile([128, H], fp16, tag="mlpn")
                    nc.scalar.copy(mn[:], mlp_ps[:])
                    mlpn.append(mn)
                mlpn_all[v] = mlpn

            def p5(v):
                gb_bc = wpool.tile([128, H], f32, tag="gb_bc")
                nc.sync.dma_start(gb_bc[:], gacb_d[v:v + 1, :].partition_broadcast(128))
                wg = wpool.tile([128, PG_X], fp16, tag="wg")
                nc.scalar.dma_start(wg[:], wp16_d[v, :, :])
                rp = wpool.tile([1, RP_X], fp16, tag="rp")
                nc.scalar.dma_start(rp[:], rp16_d[v, :, :])
                mlpn = mlpn_all[v]
                rsvh = []
                for _h in range(2):
                    rh = fe5pool.tile([128, 2 * (H + 1)], fp16, tag=f"rsv{_h}")
                    nc.sync.dma_start(
                        rh[:],
                        rsouts[v][_h][:, :].rearrange("(c p) h -> p c h", p=128))
                    rsvh.append(rh)

                featT = fgpool.tile([128, NB], fp16, tag="fTall")
                for jc in range(JC):
                    nsl = slice(jc * 128, (jc + 1) * 128)
                    rsv = rsvh[jc // 2][:, (jc % 2) * (H + 1):
                                        (jc % 2 + 1) * (H + 1)]
                    dcol = fe5pool.tile([128, 1], f32, tag="dcol")
                    nc.vector.tensor_copy(dcol[:], rsv[:, H:H + 1])
                    dinv = fe5pool.tile([128, 1], f32, tag="dinv")
                    nc.vector.reciprocal_approx_fast(out=dinv[:], in_=dcol[:])
                    featp = fe5pool.tile([128, H], f32, tag="featp")
                    nc.vector.scalar_tensor_tensor(
                        out=featp[:], in0=rsv[:, 0:H], scalar=dinv[:, 0:1],
                        in1=gb_bc[:], op0=OP.mult, op1=OP.add)
                    lk = fe5pool.tile([128, H], f32, tag="lk")
                    nc.scalar.activation(lk[:], featp[:], AF.Prelu,
                                         bias=0.0, scale=1.0, alpha=0.25)
                    feat = fe5pool.tile([128, H], fp16, tag="feat")
                    nc.vector.tensor_add(feat[:], lk[:], mlpn[jc][:])
                    t_ps = psC.tile([128, 128], fp16, tag="tp")
                    nc.tensor.transpose(t_ps[:], feat[:], ident16[:])
                    nc.scalar.copy(featT[:, nsl], t_ps[:])

                # batched K^T/V^T [o, n] for all 4 chunks
                kc_ps = psB.tile([128, NB], f32, tag="g")
                nc.tensor.matmul(kc_ps[:], wg[:, PG_KW:PG_KW + H], featT[:],
                                 start=True, stop=False)
                nc.tensor.matmul(kc_ps[:], rp[:, RP_KB:RP_KB + H],
                                 ones_nb16[:], start=False, stop=True)
                kb16 = fepool.tile([128, NB], fp16, tag="kb16")
                nc.scalar.copy(kb16[:], kc_ps[:])
                vc_ps = psB.tile([128, NB], f32, tag="g")
                nc.tensor.matmul(vc_ps[:], wg[:, PG_VW:PG_VW + H], featT[:],
                                 start=True, stop=False)
                nc.tensor.matmul(vc_ps[:], rp[:, RP_VB:RP_VB + H],
                                 ones_nb16[:], start=False, stop=True)
                vb16 = fepool.tile([128, NB], fp16, tag="vb16")
                nc.scalar.copy(vb16[:], vc_ps[:])

                # FE moment products, batched [o, NB] fp16
                kv = fepool.tile([128, NB], fp16, tag="kv")
                nc.vector.tensor_mul(kv[:], kb16[:], vb16[:])
                k2b = fepool.tile([128, NB], fp16, tag="k2b")
                nc.vector.tensor_mul(k2b[:], kb16[:], kb16[:])
                k2v = fepool.tile([128, NB], fp16, tag="k2v")
                nc.vector.tensor_mul(k2v[:], k2b[:], vb16[:])
                kpow = [None, kb16, k2b]
                kpv = {}
                for d in range(3, DFE + 1):
                    kd = fepool.tile([128, NB], fp16, tag=f"k{d}b")
                    nc.vector.tensor_mul(kd[:], kpow[d - 1][:], kb16[:])
                    kpow.append(kd)
                    kdv = fepool.tile([128, NB], fp16, tag=f"k{d}vb")
                    nc.vector.tensor_mul(kdv[:], kd[:], vb16[:])
                    kpv[d] = kdv
                mom_specs = [(0, vb16), (1, kv), (2, k2v)]
                for d in range(3, DFE + 1):
                    mom_specs.append((d, kpv[d]))
                mom_specs += [(DFE + 1, kb16), (DFE + 2, k2b)]
                for d in range(3, DFE + 1):
                    mom_specs.append((DFE + d, kpow[d]))

                aggT = fgpool.tile([128, NB], fp16, tag="aTall")
                for jc in range(JC):
                    nsl = slice(jc * 128, (jc + 1) * 128)
                    # Q [n, o] for this chunk
                    q_ps = psB.tile([128, H], f32, tag="g")
                    nc.tensor.matmul(q_ps[:], featT[:, nsl],
                                     wg[:, PG_QW:PG_QW + H],
                                     start=True, stop=False)
                    nc.tensor.matmul(q_ps[:], ones_row16[:],
                                     rp[:, RP_QB:RP_QB + H],
                                     start=False, stop=True)
                    qb_t = qpool.tile([128, H], fp16, tag="qb")
                    nc.vector.tensor_copy(qb_t[:], q_ps[:])

                    mt_ps = psB.tile([128, NMOM], f32, tag="g")
                    for col, prod in mom_specs:
                        nc.tensor.matmul(mt_ps[:, col:col + 1], prod[:, nsl],
                                         ones16c[:], start=True, stop=True)
                    mc = qpool.tile([128, NMOM], f32, tag="mcol")
                    nc.vector.tensor_mul(mc[:], mt_ps[:], fct_bc[:])

                    # FE assembly -> aggT chunk [o, n] fp16
                    pows = {1: qb_t}
                    for d in range(2, DFE + 1):
                        pd = spool.tile([128, H], fp16, tag=f"pow{d}")
                        nc.vector.tensor_mul(pd[:], pows[d - 1][:], qb_t[:])
                        pows[d] = pd
                    acc = None
                    for d in range(2, DFE + 1):
                        if acc is None:
                            acc = spool.tile([128, H], fp16, tag="accA")
                            nc.vector.tensor_scalar(
                                out=acc[:], in0=pows[d][:],
                                scalar1=mc[:, d:d + 1], scalar2=None, op0=OP.mult)
                        else:
                            nxt = spool.tile([128, H], fp16,
                                             tag="accB" if d % 2 else "accA")
                            nc.vector.scalar_tensor_tensor(
                                out=nxt[:], in0=pows[d][:], scalar=mc[:, d:d + 1],
                                in1=acc[:], op0=OP.mult, op1=OP.add)
                            acc = nxt
                    dacc = None
                    for d in range(2, DFE + 1):
                        nrow = DFE + d
                        if dacc is None:
                            dacc = spool.tile([128, H], fp16, tag="daccA")
                            nc.vector.tensor_scalar(
                                out=dacc[:], in0=pows[d][:],
                                scalar1=mc[:, nrow:nrow + 1], scalar2=None,
                                op0=OP.mult)
                        else:
                            nxt = spool.tile([128, H], fp16,
                                             tag="daccB" if d % 2 else "daccA")
                            nc.vector.scalar_tensor_tensor(
                                out=nxt[:], in0=pows[d][:],
                                scalar=mc[:, nrow:nrow + 1],
                                in1=dacc[:], op0=OP.mult, op1=OP.add)
                            dacc = nxt
                    denh = spool.tile([128, H], f32, tag="hpart")
                    nc.vector.tensor_scalar(out=denh[:], in0=qb_t[:],
                                            scalar1=mc[:, DFE + 1:DFE + 2],
                                            scalar2=float(H),
                                            op0=OP.mult, op1=OP.add)
                    den = spool.tile([128, H], f32, tag="nd")
                    nc.vector.tensor_add(den[:], denh[:], dacc[:])
                    rden = spool.tile([128, H], f32, tag="rden")
                    nc.vector.reciprocal_approx_fast(out=rden[:], in_=den[:])
                    numh = spool.tile([128, H], f32, tag="hpart")
                    nc.vector.tensor_scalar(out=numh[:], in0=qb_t[:],
                                            scalar1=mc[:, 1:2], scalar2=mc[:, 0:1],
                                            op0=OP.mult, op1=OP.add)
                    num = spool.tile([128, H], f32, tag="nd")
                    nc.vector.tensor_add(num[:], numh[:], acc[:])
                    agg = spool.tile([128, H], fp16, tag="agg")
                    nc.vector.tensor_mul(agg[:], num[:], rden[:])
                    at_ps = psC.tile([128, 128], fp16, tag="tp")
                    nc.tensor.transpose(at_ps[:], agg[:], ident16[:])
                    nc.scalar.copy(aggT[:, nsl], at_ps[:])

                # batched fc + relu; conf gate; classifier accum
                f2_ps = psB.tile([128, NB], f32, tag="g")
                nc.tensor.matmul(f2_ps[:], wg[:, PG_FW:PG_FW + H], aggT[:],
                                 start=True, stop=False)
                nc.tensor.matmul(f2_ps[:], rp[:, RP_FB:RP_FB + H],
                                 ones_nb16[:], start=False, stop=True)
                f2c = fgpool.tile([128, NB], fp16, tag="f2c")
                nc.scalar.activation(f2c[:], f2_ps[:], AF.Relu)
                cf_ps = psB.tile([1, NB], f32, tag="g")
                nc.tensor.matmul(cf_ps[:], wg[:, PG_CW:PG_CW + 1],
                                 f2c[:], start=True, stop=False)
                nc.tensor.matmul(cf_ps[:], rp[:, RP_CB:RP_CB + 1],
                                 ones_nb16[:], start=False, stop=True)
                cf_row = fepool.tile([1, NB], fp16, tag="cf_row")
                nc.scalar.copy(cf_row[:], cf_ps[:])
                cb_ps = psB.tile([128, NB], f32, tag="g")
                nc.tensor.matmul(cb_ps[:], ones_row16[:], cf_row[:],
                                 start=True, stop=True)
                gtf = fgpool.tile([128, NB], fp16, tag="gated")
                nc.vector.tensor_mul(gtf[:], f2c[:], cb_ps[:])
                nc.tensor.matmul(mm_ps[:], wg[:, PG_MMW:PG_MMW + C],
                                 gtf[:], start=(v == 0), stop=False)

            # ---------- schedule: software-pipeline the three views ----------
            warmup_cc()
            p1(0)
            p1(1)
            p1(2)
            ag_all()
            p3_adj(0)
            p3_adj(1)
            p5_mlp(0)
            p5_mlp(1)
            p5_mlp(2)
            p3(0)
            p3_adj(2)
            p3(1)
            p3(2)
            p5(0)
            p5(1)
            p5(2)

            # ---------- P9: bias + output ----------
            rp2 = wpool.tile([1, RP_X], fp16, tag="rp")
            nc.scalar.dma_start(rp2[:], rp16_d[V - 1, :, :])
            nc.tensor.matmul(mm_ps[:], rp2[:, RP_MMB:RP_MMB + C],
                             ones_nb16[:], start=False, stop=True)
            lg = fepool.tile([C, NB], fp16, tag="lg")
            nc.scalar.copy(lg[:], mm_ps[:])
            for jc in range(JC):
                lt_ps = psB.tile([128, C], fp16, tag="g")
                nc.tensor.transpose(lt_ps[:], lg[:, jc * 128:(jc + 1) * 128],
                                    ident16[0:C, 0:C])
                osb = fe5pool.tile([128, C], f32, tag="osb")
                nc.vector.tensor_copy(osb[:], lt_ps[:])
                nc.sync.dma_start(out_d[jc * 128:(jc + 1) * 128, :], osb[:])
    return nc


def _prep_inputs(inputs):
    adj = np.asarray(inputs["adj"])
    s = math.sqrt(H)
    adjT = np.ascontiguousarray(adj.transpose(0, 2, 1)).astype(np.float32)
    if ACCUM_ADD:
        adjT = (adjT - 1.0) * MASK_BIG
    adjT = adjT.astype(ml_dtypes.float8_e5m2)
    data = np.asarray(inputs["data"], dtype=np.float32)
    dataT = data.transpose(0, 2, 1)  # [V, D, N]
    f = np.float32
    wpack = np.zeros((V, 128, PK_X), np.float16)
    wpack[:, :, PK_A1] = np.asarray(inputs["a1"], f)
    wpack[:, :, PK_A2] = np.asarray(inputs["a2"], f)
    wp16 = np.zeros((V, 128, PG_X), np.float16)
    wp16[:, :, PG_QW:PG_QW + H] = np.asarray(inputs["q_w"], f) / s
    wp16[:, :, PG_KW:PG_KW + H] = np.asarray(inputs["k_w"], f)
    wp16[:, :, PG_VW:PG_VW + H] = np.asarray(inputs["v_w"], f)
    wp16[:, :, PG_FW:PG_FW + H] = np.asarray(inputs["fc_w"], f)
    wp16[:, :, PG_CW] = np.asarray(inputs["conf_w"], f).reshape(V, H)
    wp16[:, :, PG_MMW:PG_MMW + C] = np.asarray(inputs["mm_w"], f).reshape(V, H, C)
    rp16 = np.zeros((V, 1, RP_X), np.float16)
    rp16[:, 0, RP_QB:RP_QB + H] = np.asarray(inputs["q_b"], f) / s
    rp16[:, 0, RP_KB:RP_KB + H] = np.asarray(inputs["k_b"], f)
    rp16[:, 0, RP_VB:RP_VB + H] = np.asarray(inputs["v_b"], f)
    rp16[:, 0, RP_MB:RP_MB + H] = np.asarray(inputs["mlp_b"], f)
    rp16[:, 0, RP_FB:RP_FB + H] = np.asarray(inputs["fc_b"], f)
    rp16[:, 0, RP_CB] = np.asarray(inputs["conf_b"], f).reshape(V)
    rp16[:, 0, RP_MMB:RP_MMB + C] = np.asarray(inputs["mm_b"], f)[None, :]

    def chunk4(x, inner):  # [V, DC*128, inner] -> [V, 128, DC*inner]
        return np.ascontiguousarray(
            x.reshape(V, DC, 128, inner).transpose(0, 2, 1, 3)
             .reshape(V, 128, DC * inner))

    common = {
        "gacw16": chunk4(np.asarray(inputs["gac_w"], f), H).astype(np.float16),
        "mlpw16": chunk4(np.asarray(inputs["mlp_w"], f), H).astype(np.float16),
        "gacb": np.asarray(inputs["gac_b"], f),
        "wpack": wpack, "wp16": wp16, "rp16": rp16,
    }
    in_maps = []
    for c in range(NCORES):
        r0, r1 = c * NB, (c + 1) * NB
        m = dict(common)
        m["adjm"] = np.ascontiguousarray(adjT[:, r0:r1, :])
        dslice = np.ascontiguousarray(dataT[:, :, r0:r1])  # [V, D, NB]
        m["dataT16"] = chunk4(dslice, NB).astype(np.float16)
        in_maps.append(m)
    return in_maps


def kernel(**inputs):
    if "nc" not in _CACHE:
        nc = build_nc()
        nc.compile()
        _CACHE["nc"] = nc
    nc = _CACHE["nc"]
    in_maps = _prep_inputs(inputs)
    res = run_bass_kernel_spmd(nc, in_maps, list(range(NCORES)))
    out = np.concatenate([res.results[c]["out"] for c in range(NCORES)], axis=0)
    return out.astype(np.float32)


if __name__ == "__main__":
    nc = build_nc()
    print("build ok; instructions:",
          sum(len(b.instructions) for f in nc.m.functions for b in f.blocks))
    nc.compile()
    print("bacc compile ok")


# revision 14
# speedup vs baseline: 1.0460x; 1.0460x over previous
"""Trainium2 Bass kernel for DPNET (gnn_message_passing), 8-core SPMD.

Sharding: node dim N=4096 split into 8 row-blocks of 512. Each core owns the
same 512 rows for all 3 views: they serve as its block of the attention
contraction dim (j) and, after a ReduceScatter, as its output rows.

Key tricks:
  - exp(leaky(z))-masked softmax weights WITHOUT any big ACT pass:
    exp(leaky(e1_i+e2_j)) = u4_i * max(r_i*w1_j, w4_j) with r=exp(.75 e1),
    u4=exp(.25 e1), w1=exp(e2), w4=exp(.25 e2). The per-i factor u4_i scales
    numerator AND denominator of the row softmax -> cancels, so only
    W'[j,i] = adj * max(r_i*w1_j, w4_j) is needed. r is AllGathered (fp16,
    one fused collective for all 3 views); w1/w4 are per-own-row scalars.
  - adjacency mask applied via SWDGE accumulate-ADD DMA (the only CCE op
    walrus accepts): adj stored fp8e5 as {-32768, 0}, cast+added onto Wm
    during the DMA, then one relu tensor_scalar (4x-mode eligible) replaces
    the 2x-capped tensor_tensor mult. adj HBM bytes halve (1B/elem).
  - warmup collective at t=0 absorbs CC-core startup / cross-core skew.
  - softmax denominator via a ones-column in the matmul rhs; attention
    weights/partials fp16 (fp32 PSUM accum); fp16 ReduceScatter.
  - Inner FE attention exp(q_i*k_o/s) (|x|<=0.75) as a degree-DFE Taylor
    series: per-node moments m_d = sum_o k^d v / n_d = sum_o k^d via PE
    ones-reduction column matmuls; assembly with scalar_tensor_tensor.
  - all matmuls fp16 (FWL stays enabled, no fp32 PE mode switches).
  - attention PSUM banks packed 2 i-chunks wide -> half the ACT copies.
"""
import math
import numpy as np
import ml_dtypes

import concourse.bass as bass
import concourse.bacc as bacc
import concourse.mybir as mybir
import concourse.tile as tile
from concourse.bass_utils import run_bass_kernel_spmd

V, N, D, H, C = 3, 4096, 512, 128, 5
NCORES = 8
NB = N // NCORES            # 512 rows per core
JC = NB // 128              # 4 chunks of own rows
IC = N // 128               # 32 i-chunks
DC = D // 128               # 4 contraction chunks for D
DFE = 4                     # FE Taylor degree
NMOM = 2 * DFE + 1          # m_0..m_DFE, n_1..n_DFE
f32 = mybir.dt.float32
fp16 = mybir.dt.float16
fp8e5 = mybir.dt.float8e5
AF = mybir.ActivationFunctionType
OP = mybir.AluOpType
fp16np = np.float16

# fp16 wpack column layout: a1 | a2
PK_A1, PK_A2 = 0, 1
PK_X = 2
# fp16 pack: qws(128) | kw(128) | vw(128) | fcw(128) | confw | mmw(5)
PG_QW, PG_KW, PG_VW, PG_FW = 0, H, 2 * H, 3 * H
PG_CW = 4 * H
PG_MMW = 4 * H + 1
PG_X = 4 * H + 1 + C
# fp16 row pack: qbs(128) | kb(128) | vb(128) | mlpb(128) | fcb(128) | confb | mmb(5)
RP_QB, RP_KB, RP_VB, RP_MB, RP_FB = 0, H, 2 * H, 3 * H, 4 * H
RP_CB = 5 * H
RP_MMB = 5 * H + 1
RP_X = 5 * H + 1 + C

_CACHE = {}
SIM_NO_CC = False  # replace collectives with DMA stubs (for TimelineSim)
ACCUM_ADD = False  # mask via gpsimd cast+add-accum DMA + relu instead of mult
MASK_BIG = 32768.0


def build_nc():
    nc = bacc.Bacc("TRN2", target_bir_lowering=False, num_devices=NCORES)

    adjm_d = nc.dram_tensor("adjm", [V, NB, N], fp8e5, kind="ExternalInput")
    dataT16_d = nc.dram_tensor("dataT16", [V, 128, DC * NB], fp16,
                               kind="ExternalInput")
    gacw16_d = nc.dram_tensor("gacw16", [V, 128, DC * H], fp16,
                              kind="ExternalInput")
    mlpw16_d = nc.dram_tensor("mlpw16", [V, 128, DC * H], fp16,
                              kind="ExternalInput")
    gacb_d = nc.dram_tensor("gacb", [V, H], f32, kind="ExternalInput")
    wpack_d = nc.dram_tensor("wpack", [V, 128, PK_X], fp16, kind="ExternalInput")
    wp16_d = nc.dram_tensor("wp16", [V, 128, PG_X], fp16, kind="ExternalInput")
    rp16_d = nc.dram_tensor("rp16", [V, 1, RP_X], fp16, kind="ExternalInput")
    warm_d = nc.inline_tensor(np.ones((1, 4), fp16np), name="warm")

    out_d = nc.dram_tensor("out", [NB, C], f32, kind="ExternalOutput")

    fct_d = nc.inline_tensor(
        np.array([[1.0 / math.factorial(d) for d in range(DFE + 1)]
                  + [1.0 / math.factorial(d) for d in range(1, DFE + 1)]],
                 np.float32), name="fct")
    ident16_d = nc.inline_tensor(np.eye(128, dtype=fp16np), name="ident16")

    from contextlib import ExitStack
    with tile.TileContext(nc) as tc:
        with ExitStack() as stk:
            ep = lambda *a, **k: stk.enter_context(tc.tile_pool(*a, **k))
            cpool = ep(name="const", bufs=1)
            dpool = ep(name="dat", bufs=3)
            wpool = ep(name="wts", bufs=2)
            vpool = ep(name="persist", bufs=V)
            rhspool = ep(name="rhsp", bufs=V * JC)
            spool = ep(name="scratch", bufs=3)
            e1pool = ep(name="e1bp", bufs=2)
            wmpool = ep(name="wmp", bufs=8)
            apool = ep(name="adjp", bufs=5)
            a4pool = ep(name="att4p", bufs=4)
            fepool = ep(name="fe2", bufs=2)
            fgpool = ep(name="feg", bufs=2)
            fe5pool = ep(name="fe5", bufs=3)
            qpool = ep(name="qp", bufs=4)
            mlppool = ep(name="mlpp", bufs=V * JC)
            psA = ep(name="psA", bufs=2, space="PSUM")
            psB = ep(name="psB", bufs=3, space="PSUM")
            psC = ep(name="psC", bufs=2, space="PSUM")
            psM = ep(name="psM", bufs=1, space="PSUM")
            drpool = ep(name="dram", bufs=1, space="DRAM")

            # ---------- constants ----------
            ident16 = cpool.tile([128, 128], fp16, tag="c1")
            nc.sync.dma_start(ident16[:], ident16_d[:])
            ones16c = cpool.tile([128, 1], fp16, tag="c3")
            nc.vector.memset(ones16c[:], 1.0)
            ones_row16 = cpool.tile([1, 128], fp16, tag="c5")
            nc.vector.memset(ones_row16[:], 1.0)
            fct_bc = cpool.tile([128, NMOM], f32, tag="c7")
            nc.sync.dma_start(fct_bc[:], fct_d[0:1, :].partition_broadcast(128))
            ones_nb16 = cpool.tile([1, NB], fp16, tag="c8")
            nc.vector.memset(ones_nb16[:], 1.0)

            partials, rsouts = [], []
            agi_all = drpool.tile([V, NB], fp16, tag="agi")
            ago_all = drpool.tile([NCORES, V, NB], fp16,
                                  addr_space="Local" if SIM_NO_CC else "Shared",
                                  tag="ago")
            warm_o = drpool.tile([NCORES, 1, 4], fp16,
                                 addr_space="Local" if SIM_NO_CC else "Shared",
                                 tag="warm_o")
            for _pv in range(V):
                pt = drpool.tile([NCORES, NB, H + 1], fp16, tag=f"part{_pv}")
                partials.append(pt)
                rt_ = drpool.tile([NB, H + 1], fp16, tag=f"rsout{_pv}")
                rsouts.append(rt_)

            # warmup collective: absorbs CC-core startup / cross-core skew
            def warmup_cc():
                if not SIM_NO_CC:
                    nc.gpsimd.collective_compute(
                        "AllGather", OP.bypass,
                        replica_groups=[list(range(NCORES))],
                        ins=[warm_d[:, :]], outs=[warm_o.opt()])

            # ---------- P1 per view: hT, r (+gather), w1/w4, rhs tiles ----------
            rhs_sb = [[None] * JC for _ in range(V)]
            w1_sb = [None] * V
            w4_sb = [None] * V
            dt16_all = [None] * V

            def p1(v):
                dt16 = dpool.tile([128, DC * NB], fp16, tag="dt16")
                nc.sync.dma_start(dt16[:], dataT16_d[v, :, :])
                dt16_all[v] = dt16
                gw16 = wpool.tile([128, DC * H], fp16, tag="gw16")
                nc.scalar.dma_start(gw16[:], gacw16_d[v, :, :])
                wp = wpool.tile([128, PK_X], fp16, tag="wp")
                nc.scalar.dma_start(wp[:], wpack_d[v, :, :])
                # hT = (data @ gac_w).T : lhsT=gw chunk [d,H], rhs=dataT chunk
                hT_ps = psM.tile([128, NB], f32, tag="mm")
                for dc in range(DC):
                    nc.tensor.matmul(
                        hT_ps[:], gw16[:, dc * H:(dc + 1) * H],
                        dt16[:, dc * NB:(dc + 1) * NB],
                        start=(dc == 0), stop=(dc == DC - 1))
                hT = cpool.tile([128, NB], fp16, tag="hT")
                nc.scalar.copy(hT[:], hT_ps[:])
                e1_ps = psB.tile([1, NB], f32, tag="g")
                nc.tensor.matmul(e1_ps[:], wp[:, PK_A1:PK_A1 + 1], hT[:],
                                 start=True, stop=True)
                # r = exp(0.75*e1) (the u4=exp(.25 e1) factor cancels in the
                # softmax normalization, so only r is gathered)
                rrow = cpool.tile([1, NB], fp16, tag="rrow")
                nc.scalar.activation(rrow[:], e1_ps[:], AF.Exp,
                                     bias=0.0, scale=0.75)
                nc.scalar.dma_start(agi_all[v:v + 1, :], rrow[:])
                e2c = vpool.tile([128, JC], f32, tag="e2c")
                for jc in range(JC):
                    e2_ps = psB.tile([128, 1], f32, tag="g")
                    nc.tensor.matmul(
                        e2_ps[:], hT[:, jc * 128:(jc + 1) * 128],
                        wp[:, PK_A2:PK_A2 + 1], start=True, stop=True)
                    nc.scalar.copy(e2c[:, jc:jc + 1], e2_ps[:])
                w1c = vpool.tile([128, JC], f32, tag="w1c")
                nc.scalar.activation(w1c[:], e2c[:], AF.Exp, bias=0.0, scale=1.0)
                w4c = vpool.tile([128, JC], f32, tag="w4c")
                nc.scalar.activation(w4c[:], e2c[:], AF.Exp, bias=0.0, scale=0.25)
                w1_sb[v] = w1c
                w4_sb[v] = w4c
                for jc in range(JC):
                    t_ps = psC.tile([128, 128], fp16, tag="tp")
                    nc.tensor.transpose(
                        t_ps[:], hT[:, jc * 128:(jc + 1) * 128], ident16[:])
                    rt = rhspool.tile([128, H + 1], fp16, tag="rhs")
                    nc.scalar.copy(rt[:, 0:H], t_ps[:])
                    nc.vector.memset(rt[:, H:H + 1], 1.0)
                    rhs_sb[v][jc] = rt

            # single fused AllGather for all three views' r rows
            def ag_all():
                if SIM_NO_CC:
                    nc.sync.dma_start(
                        ago_all[:, :, :],
                        agi_all[:, :].partition_broadcast(NCORES))
                else:
                    nc.gpsimd.collective_compute(
                        "AllGather", OP.bypass,
                        replica_groups=[list(range(NCORES))],
                        ins=[agi_all.opt()], outs=[ago_all.opt()])

            # ---------- P3/P4 per view: attention + partials + RS ----------
            madj_all = {}

            def p3_adj(v):
                # AG-independent: prefetch+cast the adjacency tiles early
                for jc in range(JC):
                    madj = apool.tile([128, N], fp16, tag="madj")
                    nc.gpsimd.dma_start(
                        madj[:], adjm_d[v, jc * 128:(jc + 1) * 128, :])
                    madj_all[(v, jc)] = madj

            def p3(v):
                rb = e1pool.tile([128, N], fp16, tag="rb")
                nc.sync.dma_start(
                    rb[:].rearrange("p (k r) -> p k r", k=NCORES),
                    ago_all[:, v:v + 1, :].rearrange("k o r -> o k r")
                    .partition_broadcast(128))
                wms = []
                for jc in range(JC):
                    Wm = wmpool.tile([128, N], fp16, tag="Wm")
                    nc.vector.tensor_scalar(
                        out=Wm[:], in0=rb[:],
                        scalar1=w1_sb[v][:, jc:jc + 1],
                        scalar2=w4_sb[v][:, jc:jc + 1],
                        op0=OP.mult, op1=OP.max)
                    madj = madj_all.pop((v, jc))
                    nc.vector.tensor_tensor(Wm[:], Wm[:], madj[:], OP.mult)
                    
                    wms.append(Wm)
                # attention: PSUM banks packed 2 i-chunks wide; one fp16 att2
                # copy + one partials DMA per pair
                for gp in range(IC // 2):
                    att_ps = psA.tile([128, 2 * (H + 1)], f32, tag="att")
                    for c2 in range(2):
                        g = gp * 2 + c2
                        osl = slice(c2 * (H + 1), (c2 + 1) * (H + 1))
                        for jc in range(JC):
                            nc.tensor.matmul(
                                att_ps[:, osl],
                                wms[jc][:, g * 128:(g + 1) * 128],
                                rhs_sb[v][jc][:],
                                start=(jc == 0), stop=(jc == JC - 1))
                    att2 = a4pool.tile([128, 2 * (H + 1)], fp16, tag="att4")
                    nc.scalar.copy(att2[:], att_ps[:])
                    ko, ro = gp // 2, (gp % 2) * 256
                    nc.sync.dma_start(
                        partials[v][ko, ro:ro + 256, :]
                        .rearrange("(c p) h -> p c h", p=128),
                        att2[:].rearrange("p (c h) -> p c h", c=2))
                if SIM_NO_CC:
                    nc.sync.dma_start(rsouts[v][:, :], partials[v][0, :, :])
                else:
                    nc.gpsimd.collective_compute(
                        "ReduceScatter", OP.add,
                        replica_groups=[list(range(NCORES))],
                        ins=[partials[v].opt()], outs=[rsouts[v].opt()])

            # ---------- P5..P9 per view: fully per-chunk pipelined ----------
            mm_ps = psM.tile([C, NB], f32, tag="mm")

            mlpn_all = [None] * V

            def p5_mlp(v):
                # mlp branch: independent of the ReduceScatter -> compute
                # before the collectives to fill the startup dead zone
                mw16 = wpool.tile([128, DC * H], fp16, tag="mw16")
                nc.scalar.dma_start(mw16[:], mlpw16_d[v, :, :])
                rpm = wpool.tile([1, RP_X], fp16, tag="rpm")
                nc.scalar.dma_start(rpm[:], rp16_d[v, :, :])
                dt16 = dt16_all[v]
                mlpn = []
                for jc in range(JC):
                    mlp_ps = psB.tile([128, H], f32, tag="g")
                    for dc in range(DC):
                        nc.tensor.matmul(
                            mlp_ps[:],
                            dt16[:, dc * NB + jc * 128:dc * NB + (jc + 1) * 128],
                            mw16[:, dc * H:(dc + 1) * H],
                            start=(dc == 0), stop=False)
                    nc.tensor.matmul(mlp_ps[:], ones_row16[:],
                                     rpm[:, RP_MB:RP_MB + H],
                                     start=False, stop=True)
                    mn = mlppool.tile([128, H], fp16, tag="mlpn")
                    nc.scalar.copy(mn[:], mlp_ps[:])
                    mlpn.append(mn)
                mlpn_all[v] = mlpn

            def p5(v):
                gb_bc = wpool.tile([128, H], f32, tag="gb_bc")
                nc.sync.dma_start(gb_bc[:], gacb_d[v:v + 1, :].partition_broadcast(128))
                wg = wpool.tile([128, PG_X], fp16, tag="wg")
                nc.scalar.dma_start(wg[:], wp16_d[v, :, :])
                rp = wpool.tile([1, RP_X], fp16, tag="rp")
                nc.scalar.dma_start(rp[:], rp16_d[v, :, :])
                mlpn = mlpn_all[v]
                rsv4 = fe5pool.tile([128, JC * (H + 1)], fp16, tag="rsv4")
                nc.sync.dma_start(
                    rsv4[:],
                    rsouts[v][:, :].rearrange("(c p) h -> p c h", p=128))

                featT = fgpool.tile([128, NB], fp16, tag="fTall")
                for jc in range(JC):
                    nsl = slice(jc * 128, (jc + 1) * 128)
                    rsv = rsv4[:, jc * (H + 1):(jc + 1) * (H + 1)]
                    dcol = fe5pool.tile([128, 1], f32, tag="dcol")
                    nc.vector.tensor_copy(dcol[:], rsv[:, H:H + 1])
                    dinv = fe5pool.tile([128, 1], f32, tag="dinv")
                    nc.vector.reciprocal_approx_fast(out=dinv[:], in_=dcol[:])
                    featp = fe5pool.tile([128, H], f32, tag="featp")
                    nc.vector.scalar_tensor_tensor(
                        out=featp[:], in0=rsv[:, 0:H], scalar=dinv[:, 0:1],
                        in1=gb_bc[:], op0=OP.mult, op1=OP.add)
                    lk = fe5pool.tile([128, H], f32, tag="lk")
                    nc.scalar.activation(lk[:], featp[:], AF.Prelu,
                                         bias=0.0, scale=1.0, alpha=0.25)
                    feat = fe5pool.tile([128, H], fp16, tag="feat")
                    nc.vector.tensor_add(feat[:], lk[:], mlpn[jc][:])
                    t_ps = psC.tile([128, 128], fp16, tag="tp")
                    nc.tensor.transpose(t_ps[:], feat[:], ident16[:])
                    nc.scalar.copy(featT[:, nsl], t_ps[:])

                # batched K^T/V^T [o, n] for all 4 chunks
                kc_ps = psB.tile([128, NB], f32, tag="g")
                nc.tensor.matmul(kc_ps[:], wg[:, PG_KW:PG_KW + H], featT[:],
                                 start=True, stop=False)
                nc.tensor.matmul(kc_ps[:], rp[:, RP_KB:RP_KB + H],
                                 ones_nb16[:], start=False, stop=True)
                kb16 = fepool.tile([128, NB], fp16, tag="kb16")
                nc.scalar.copy(kb16[:], kc_ps[:])
                vc_ps = psB.tile([128, NB], f32, tag="g")
                nc.tensor.matmul(vc_ps[:], wg[:, PG_VW:PG_VW + H], featT[:],
                                 start=True, stop=False)
                nc.tensor.matmul(vc_ps[:], rp[:, RP_VB:RP_VB + H],
                                 ones_nb16[:], start=False, stop=True)
                vb16 = fepool.tile([128, NB], fp16, tag="vb16")
                nc.scalar.copy(vb16[:], vc_ps[:])

                # FE moment products, batched [o, NB] fp16
                kv = fepool.tile([128, NB], fp16, tag="kv")
                nc.vector.tensor_mul(kv[:], kb16[:], vb16[:])
                k2b = fepool.tile([128, NB], fp16, tag="k2b")
                nc.vector.tensor_mul(k2b[:], kb16[:], kb16[:])
                k2v = fepool.tile([128, NB], fp16, tag="k2v")
                nc.vector.tensor_mul(k2v[:], k2b[:], vb16[:])
                kpow = [None, kb16, k2b]
                kpv = {}
                for d in range(3, DFE + 1):
                    kd = fepool.tile([128, NB], fp16, tag=f"k{d}b")
                    nc.vector.tensor_mul(kd[:], kpow[d - 1][:], kb16[:])
                    kpow.append(kd)
                    kdv = fepool.tile([128, NB], fp16, tag=f"k{d}vb")
                    nc.vector.tensor_mul(kdv[:], kd[:], vb16[:])
                    kpv[d] = kdv
                mom_specs = [(0, vb16), (1, kv), (2, k2v)]
                for d in range(3, DFE + 1):
                    mom_specs.append((d, kpv[d]))
                mom_specs += [(DFE + 1, kb16), (DFE + 2, k2b)]
                for d in range(3, DFE + 1):
                    mom_specs.append((DFE + d, kpow[d]))

                aggT = fgpool.tile([128, NB], fp16, tag="aTall")
                for jc in range(JC):
                    nsl = slice(jc * 128, (jc + 1) * 128)
                    # Q [n, o] for this chunk
                    q_ps = psB.tile([128, H], f32, tag="g")
                    nc.tensor.matmul(q_ps[:], featT[:, nsl],
                                     wg[:, PG_QW:PG_QW + H],
                                     start=True, stop=False)
                    nc.tensor.matmul(q_ps[:], ones_row16[:],
                                     rp[:, RP_QB:RP_QB + H],
                                     start=False, stop=True)
                    qb_t = qpool.tile([128, H], fp16, tag="qb")
                    nc.vector.tensor_copy(qb_t[:], q_ps[:])

                    mt_ps = psB.tile([128, NMOM], f32, tag="g")
                    for col, prod in mom_specs:
                        nc.tensor.matmul(mt_ps[:, col:col + 1], prod[:, nsl],
                                         ones16c[:], start=True, stop=True)
                    mc = qpool.tile([128, NMOM], f32, tag="mcol")
                    nc.vector.tensor_mul(mc[:], mt_ps[:], fct_bc[:])

                    # FE assembly -> aggT chunk [o, n] fp16
                    pows = {1: qb_t}
                    for d in range(2, DFE + 1):
                        pd = spool.tile([128, H], fp16, tag=f"pow{d}")
                        nc.vector.tensor_mul(pd[:], pows[d - 1][:], qb_t[:])
                        pows[d] = pd
                    acc = None
                    for d in range(2, DFE + 1):
                        if acc is None:
                            acc = spool.tile([128, H], fp16, tag="accA")
                            nc.vector.tensor_scalar(
                                out=acc[:], in0=pows[d][:],
                                scalar1=mc[:, d:d + 1], scalar2=None, op0=OP.mult)
                        else:
                            nxt = spool.tile([128, H], fp16,
                                             tag="accB" if d % 2 else "accA")
                            nc.vector.scalar_tensor_tensor(
                                out=nxt[:], in0=pows[d][:], scalar=mc[:, d:d + 1],
                                in1=acc[:], op0=OP.mult, op1=OP.add)
                            acc = nxt
                    dacc = None
                    for d in range(2, DFE + 1):
                        nrow = DFE + d
                        if dacc is None:
                            dacc = spool.tile([128, H], fp16, tag="daccA")
                            nc.vector.tensor_scalar(
                                out=dacc[:], in0=pows[d][:],
                                scalar1=mc[:, nrow:nrow + 1], scalar2=None,
                                op0=OP.mult)
                        else:
                            nxt = spool.tile([128, H], fp16,
                                             tag="daccB" if d % 2 else "daccA")
                            nc.vector.scalar_tensor_tensor(
                                out=nxt[:], in0=pows[d][:],
                                scalar=mc[:, nrow:nrow + 1],
                                in1=dacc[:], op0=OP.mult, op1=OP.add)
                            dacc = nxt
                    denh = spool.tile([128, H], f32, tag="hpart")
                    nc.vector.tensor_scalar(out=denh[:], in0=qb_t[:],
                                            scalar1=mc[:, DFE + 1:DFE + 2],
                                            scalar2=float(H),
                                            op0=OP.mult, op1=OP.add)
                    den = spool.tile([128, H], f32, tag="nd")
                    nc.vector.tensor_add(den[:], denh[:], dacc[:])
                    rden = spool.tile([128, H], f32, tag="rden")
                    nc.vector.reciprocal_approx_fast(out=rden[:], in_=den[:])
                    numh = spool.tile([128, H], f32, tag="hpart")
                    nc.vector.tensor_scalar(out=numh[:], in0=qb_t[:],
                                            scalar1=mc[:, 1:2], scalar2=mc[:, 0:1],
                                            op0=OP.mult, op1=OP.add)
                    num = spool.tile([128, H], f32, tag="nd")
                    nc.vector.tensor_add(num[:], numh[:], acc[:])
                    agg = spool.tile([128, H], fp16, tag="agg")
                    nc.vector.tensor_mul(agg[:], num[:], rden[:])
                    at_ps = psC.tile([128, 128], fp16, tag="tp")
                    nc.tensor.transpose(at_ps[:], agg[:], ident16[:])
                    nc.scalar.copy(aggT[:, nsl], at_ps[:])

                # batched fc + relu; conf gate; classifier accum
                f2_ps = psB.tile([128, NB], f32, tag="g")
                nc.tensor.matmul(f2_ps[:], wg[:, PG_FW:PG_FW + H], aggT[:],
                                 start=True, stop=False)
                nc.tensor.matmul(f2_ps[:], rp[:, RP_FB:RP_FB + H],
                                 ones_nb16[:], start=False, stop=True)
                f2c = fgpool.tile([128, NB], fp16, tag="f2c")
                nc.scalar.activation(f2c[:], f2_ps[:], AF.Relu)
                cf_ps = psB.tile([1, NB], f32, tag="g")
                nc.tensor.matmul(cf_ps[:], wg[:, PG_CW:PG_CW + 1],
                                 f2c[:], start=True, stop=False)
                nc.tensor.matmul(cf_ps[:], rp[:, RP_CB:RP_CB + 1],
                                 ones_nb16[:], start=False, stop=True)
                cf_row = fepool.tile([1, NB], fp16, tag="cf_row")
                nc.scalar.copy(cf_row[:], cf_ps[:])
                cb_ps = psB.tile([128, NB], f32, tag="g")
                nc.tensor.matmul(cb_ps[:], ones_row16[:], cf_row[:],
                                 start=True, stop=True)
                gtf = fgpool.tile([128, NB], fp16, tag="gated")
                nc.vector.tensor_mul(gtf[:], f2c[:], cb_ps[:])
                nc.tensor.matmul(mm_ps[:], wg[:, PG_MMW:PG_MMW + C],
                                 gtf[:], start=(v == 0), stop=False)

            # ---------- schedule: software-pipeline the three views ----------
            warmup_cc()
            p1(0)
            p1(1)
            p1(2)
            ag_all()
            p3_adj(0)
            p3_adj(1)
            p5_mlp(0)
            p5_mlp(1)
            p5_mlp(2)
            p3(0)
            p3_adj(2)
            p3(1)
            p3(2)
            p5(0)
            p5(1)
            p5(2)

            # ---------- P9: bias + output ----------
            rp2 = wpool.tile([1, RP_X], fp16, tag="rp")
            nc.scalar.dma_start(rp2[:], rp16_d[V - 1, :, :])
            nc.tensor.matmul(mm_ps[:], rp2[:, RP_MMB:RP_MMB + C],
                             ones_nb16[:], start=False, stop=True)
            lg = fepool.tile([C, NB], fp16, tag="lg")
            nc.scalar.copy(lg[:], mm_ps[:])
            for jc in range(JC):
                lt_ps = psB.tile([128, C], fp16, tag="g")
                nc.tensor.transpose(lt_ps[:], lg[:, jc * 128:(jc + 1) * 128],
                                    ident16[0:C, 0:C])
                osb = fe5pool.tile([128, C], f32, tag="osb")
                nc.vector.tensor_copy(osb[:], lt_ps[:])
                nc.sync.dma_start(out_d[jc * 128:(jc + 1) * 128, :], osb[:])
    return nc


def _prep_inputs(inputs):
    adj = np.asarray(inputs["adj"])
    s = math.sqrt(H)
    adjT = np.ascontiguousarray(adj.transpose(0, 2, 1)).astype(np.float32)
    if ACCUM_ADD:
        adjT = (adjT - 1.0) * MASK_BIG
    adjT = adjT.astype(ml_dtypes.float8_e5m2)
    data = np.asarray(inputs["data"], dtype=np.float32)
    dataT = data.transpose(0, 2, 1)  # [V, D, N]
    f = np.float32
    wpack = np.zeros((V, 128, PK_X), np.float16)
    wpack[:, :, PK_A1] = np.asarray(inputs["a1"], f)
    wpack[:, :, PK_A2] = np.asarray(inputs["a2"], f)
    wp16 = np.zeros((V, 128, PG_X), np.float16)
    wp16[:, :, PG_QW:PG_QW + H] = np.asarray(inputs["q_w"], f) / s
    wp16[:, :, PG_KW:PG_KW + H] = np.asarray(inputs["k_w"], f)
    wp16[:, :, PG_VW:PG_VW + H] = np.asarray(inputs["v_w"], f)
    wp16[:, :, PG_FW:PG_FW + H] = np.asarray(inputs["fc_w"], f)
    wp16[:, :, PG_CW] = np.asarray(inputs["conf_w"], f).reshape(V, H)
    wp16[:, :, PG_MMW:PG_MMW + C] = np.asarray(inputs["mm_w"], f).reshape(V, H, C)
    rp16 = np.zeros((V, 1, RP_X), np.float16)
    rp16[:, 0, RP_QB:RP_QB + H] = np.asarray(inputs["q_b"], f) / s
    rp16[:, 0, RP_KB:RP_KB + H] = np.asarray(inputs["k_b"], f)
    rp16[:, 0, RP_VB:RP_VB + H] = np.asarray(inputs["v_b"], f)
    rp16[:, 0, RP_MB:RP_MB + H] = np.asarray(inputs["mlp_b"], f)
    rp16[:, 0, RP_FB:RP_FB + H] = np.asarray(inputs["fc_b"], f)
    rp16[:, 0, RP_CB] = np.asarray(inputs["conf_b"], f).reshape(V)
    rp16[:, 0, RP_MMB:RP_MMB + C] = np.asarray(inputs["mm_b"], f)[None, :]

    def chunk4(x, inner):  # [V, DC*128, inner] -> [V, 128, DC*inner]
        return np.ascontiguousarray(
            x.reshape(V, DC, 128, inner).transpose(0, 2, 1, 3)
             .reshape(V, 128, DC * inner))

    common = {
        "gacw16": chunk4(np.asarray(inputs["gac_w"], f), H).astype(np.float16),
        "mlpw16": chunk4(np.asarray(inputs["mlp_w"], f), H).astype(np.float16),
        "gacb": np.asarray(inputs["gac_b"], f),
        "wpack": wpack, "wp16": wp16, "rp16": rp16,
    }
    in_maps = []
    for c in range(NCORES):
        r0, r1 = c * NB, (c + 1) * NB
        m = dict(common)
        m["adjm"] = np.ascontiguousarray(adjT[:, r0:r1, :])
        dslice = np.ascontiguousarray(dataT[:, :, r0:r1])  # [V, D, NB]
        m["dataT16"] = chunk4(dslice, NB).astype(np.float16)
        in_maps.append(m)
    return in_maps


def kernel(**inputs):
    if "nc" not in _CACHE:
        nc = build_nc()
        nc.compile()
        _CACHE["nc"] = nc
    nc = _CACHE["nc"]
    in_maps = _prep_inputs(inputs)
    res = run_bass_kernel_spmd(nc, in_maps, list(range(NCORES)))
    out = np.concatenate([res.results[c]["out"] for c in range(NCORES)], axis=0)
    return out.astype(np.float32)


if __name__ == "__main__":
    nc = build_nc()
    print("build ok; instructions:",
          sum(len(b.instructions) for f in nc.m.functions for b in f.blocks))
    nc.compile()
    print("bacc compile ok")


# revision 15
# speedup vs baseline: 1.0499x; 1.0038x over previous
"""Trainium2 Bass kernel for DPNET (gnn_message_passing), 8-core SPMD.

Sharding: node dim N=4096 split into 8 row-blocks of 512. Each core owns the
same 512 rows for all 3 views: they serve as its block of the attention
contraction dim (j) and, after a ReduceScatter, as its output rows.

Key tricks:
  - exp(leaky(z))-masked softmax weights WITHOUT any big ACT pass:
    exp(leaky(e1_i+e2_j)) = u4_i * max(r_i*w1_j, w4_j) with r=exp(.75 e1),
    u4=exp(.25 e1), w1=exp(e2), w4=exp(.25 e2). The per-i factor u4_i scales
    numerator AND denominator of the row softmax -> cancels, so only
    W'[j,i] = adj * max(r_i*w1_j, w4_j) is needed. r is AllGathered (fp16,
    one fused collective for all 3 views); w1/w4 are per-own-row scalars.
  - adjacency mask applied via SWDGE accumulate-ADD DMA (the only CCE op
    walrus accepts): adj stored fp8e5 as {-32768, 0}, cast+added onto Wm
    during the DMA, then one relu tensor_scalar (4x-mode eligible) replaces
    the 2x-capped tensor_tensor mult. adj HBM bytes halve (1B/elem).
  - warmup collective at t=0 absorbs CC-core startup / cross-core skew.
  - softmax denominator via a ones-column in the matmul rhs; attention
    weights/partials fp16 (fp32 PSUM accum); fp16 ReduceScatter.
  - Inner FE attention exp(q_i*k_o/s) (|x|<=0.75) as a degree-DFE Taylor
    series: per-node moments m_d = sum_o k^d v / n_d = sum_o k^d via PE
    ones-reduction column matmuls; assembly with scalar_tensor_tensor.
  - all matmuls fp16 (FWL stays enabled, no fp32 PE mode switches).
  - attention PSUM banks packed 2 i-chunks wide -> half the ACT copies.
"""
import math
import numpy as np
import ml_dtypes

import concourse.bass as bass
import concourse.bacc as bacc
import concourse.mybir as mybir
import concourse.tile as tile
from concourse.bass_utils import run_bass_kernel_spmd

V, N, D, H, C = 3, 4096, 512, 128, 5
NCORES = 8
NB = N // NCORES            # 512 rows per core
JC = NB // 128              # 4 chunks of own rows
IC = N // 128               # 32 i-chunks
DC = D // 128               # 4 contraction chunks for D
DFE = 4                     # FE Taylor degree
NMOM = 2 * DFE + 1          # m_0..m_DFE, n_1..n_DFE
f32 = mybir.dt.float32
fp16 = mybir.dt.float16
fp8e5 = mybir.dt.float8e5
AF = mybir.ActivationFunctionType
OP = mybir.AluOpType
fp16np = np.float16

# fp16 wpack column layout: a1 | a2
PK_A1, PK_A2 = 0, 1
PK_X = 2
# fp16 pack: qws(128) | kw(128) | vw(128) | fcw(128) | confw | mmw(5)
PG_QW, PG_KW, PG_VW, PG_FW = 0, H, 2 * H, 3 * H
PG_CW = 4 * H
PG_MMW = 4 * H + 1
PG_X = 4 * H + 1 + C
# fp16 row pack: qbs(128) | kb(128) | vb(128) | mlpb(128) | fcb(128) | confb | mmb(5)
RP_QB, RP_KB, RP_VB, RP_MB, RP_FB = 0, H, 2 * H, 3 * H, 4 * H
RP_CB = 5 * H
RP_MMB = 5 * H + 1
RP_X = 5 * H + 1 + C

_CACHE = {}
SIM_NO_CC = False  # replace collectives with DMA stubs (for TimelineSim)
ACCUM_ADD = False  # mask via gpsimd cast+add-accum DMA + relu instead of mult
MASK_BIG = 32768.0


def build_nc():
    nc = bacc.Bacc("TRN2", target_bir_lowering=False, num_devices=NCORES)

    adjm_d = nc.dram_tensor("adjm", [V, NB, N], fp8e5, kind="ExternalInput")
    dataT16_d = nc.dram_tensor("dataT16", [V, 128, DC * NB], fp16,
                               kind="ExternalInput")
    gacw16_d = nc.dram_tensor("gacw16", [V, 128, DC * H], fp16,
                              kind="ExternalInput")
    mlpw16_d = nc.dram_tensor("mlpw16", [V, 128, DC * H], fp16,
                              kind="ExternalInput")
    gacb_d = nc.dram_tensor("gacb", [V, H], f32, kind="ExternalInput")
    wpack_d = nc.dram_tensor("wpack", [V, 128, PK_X], fp16, kind="ExternalInput")
    wp16_d = nc.dram_tensor("wp16", [V, 128, PG_X], fp16, kind="ExternalInput")
    rp16_d = nc.dram_tensor("rp16", [V, 1, RP_X], fp16, kind="ExternalInput")
    warm_d = nc.inline_tensor(np.ones((1, 4), fp16np), name="warm")

    out_d = nc.dram_tensor("out", [NB, C], f32, kind="ExternalOutput")

    fct_d = nc.inline_tensor(
        np.array([[1.0 / math.factorial(d) for d in range(DFE + 1)]
                  + [1.0 / math.factorial(d) for d in range(1, DFE + 1)]],
                 np.float32), name="fct")
    ident16_d = nc.inline_tensor(np.eye(128, dtype=fp16np), name="ident16")

    from contextlib import ExitStack
    with tile.TileContext(nc) as tc:
        with ExitStack() as stk:
            ep = lambda *a, **k: stk.enter_context(tc.tile_pool(*a, **k))
            cpool = ep(name="const", bufs=1)
            dpool = ep(name="dat", bufs=3)
            wpool = ep(name="wts", bufs=2)
            vpool = ep(name="persist", bufs=V)
            rhspool = ep(name="rhsp", bufs=V * JC)
            spool = ep(name="scratch", bufs=3)
            e1pool = ep(name="e1bp", bufs=2)
            wmpool = ep(name="wmp", bufs=8)
            apool = ep(name="adjp", bufs=5)
            a4pool = ep(name="att4p", bufs=4)
            fepool = ep(name="fe2", bufs=2)
            fgpool = ep(name="feg", bufs=2)
            fe5pool = ep(name="fe5", bufs=3)
            qpool = ep(name="qp", bufs=4)
            mlppool = ep(name="mlpp", bufs=V * JC)
            psA = ep(name="psA", bufs=2, space="PSUM")
            psB = ep(name="psB", bufs=3, space="PSUM")
            psC = ep(name="psC", bufs=2, space="PSUM")
            psM = ep(name="psM", bufs=1, space="PSUM")
            drpool = ep(name="dram", bufs=1, space="DRAM")

            # ---------- constants ----------
            ident16 = cpool.tile([128, 128], fp16, tag="c1")
            nc.sync.dma_start(ident16[:], ident16_d[:])
            ones16c = cpool.tile([128, 1], fp16, tag="c3")
            nc.vector.memset(ones16c[:], 1.0)
            ones_row16 = cpool.tile([1, 128], fp16, tag="c5")
            nc.vector.memset(ones_row16[:], 1.0)
            fct_bc = cpool.tile([128, NMOM], f32, tag="c7")
            nc.sync.dma_start(fct_bc[:], fct_d[0:1, :].partition_broadcast(128))
            ones_nb16 = cpool.tile([1, NB], fp16, tag="c8")
            nc.vector.memset(ones_nb16[:], 1.0)

            partials, rsouts = [], []
            agi_all = drpool.tile([V, NB], fp16, tag="agi")
            ago_all = drpool.tile([NCORES, V, NB], fp16,
                                  addr_space="Local" if SIM_NO_CC else "Shared",
                                  tag="ago")
            warm_o = drpool.tile([NCORES, 1, 4], fp16,
                                 addr_space="Local" if SIM_NO_CC else "Shared",
                                 tag="warm_o")
            for _pv in range(V):
                pt = drpool.tile([NCORES, NB, H + 1], fp16, tag=f"part{_pv}")
                partials.append(pt)
                rt_ = drpool.tile([NB, H + 1], fp16, tag=f"rsout{_pv}")
                rsouts.append(rt_)

            # warmup collective: absorbs CC-core startup / cross-core skew
            def warmup_cc():
                if not SIM_NO_CC:
                    nc.gpsimd.collective_compute(
                        "AllGather", OP.bypass,
                        replica_groups=[list(range(NCORES))],
                        ins=[warm_d[:, :]], outs=[warm_o.opt()])

            # ---------- P1 per view: hT, r (+gather), w1/w4, rhs tiles ----------
            rhs_sb = [[None] * JC for _ in range(V)]
            w1_sb = [None] * V
            w4_sb = [None] * V
            dt16_all = [None] * V

            def p1(v):
                dt16 = dpool.tile([128, DC * NB], fp16, tag="dt16")
                nc.sync.dma_start(dt16[:], dataT16_d[v, :, :])
                dt16_all[v] = dt16
                gw16 = wpool.tile([128, DC * H], fp16, tag="gw16")
                nc.scalar.dma_start(gw16[:], gacw16_d[v, :, :])
                wp = wpool.tile([128, PK_X], fp16, tag="wp")
                nc.scalar.dma_start(wp[:], wpack_d[v, :, :])
                # hT = (data @ gac_w).T : lhsT=gw chunk [d,H], rhs=dataT chunk
                hT_ps = psM.tile([128, NB], f32, tag="mm")
                for dc in range(DC):
                    nc.tensor.matmul(
                        hT_ps[:], gw16[:, dc * H:(dc + 1) * H],
                        dt16[:, dc * NB:(dc + 1) * NB],
                        start=(dc == 0), stop=(dc == DC - 1))
                hT = cpool.tile([128, NB], fp16, tag="hT")
                nc.scalar.copy(hT[:], hT_ps[:])
                e1_ps = psB.tile([1, NB], f32, tag="g")
                nc.tensor.matmul(e1_ps[:], wp[:, PK_A1:PK_A1 + 1], hT[:],
                                 start=True, stop=True)
                # r = exp(0.75*e1) (the u4=exp(.25 e1) factor cancels in the
                # softmax normalization, so only r is gathered)
                rrow = cpool.tile([1, NB], fp16, tag="rrow")
                nc.scalar.activation(rrow[:], e1_ps[:], AF.Exp,
                                     bias=0.0, scale=0.75)
                nc.scalar.dma_start(agi_all[v:v + 1, :], rrow[:])
                e2c = vpool.tile([128, JC], f32, tag="e2c")
                for jc in range(JC):
                    e2_ps = psB.tile([128, 1], f32, tag="g")
                    nc.tensor.matmul(
                        e2_ps[:], hT[:, jc * 128:(jc + 1) * 128],
                        wp[:, PK_A2:PK_A2 + 1], start=True, stop=True)
                    nc.scalar.copy(e2c[:, jc:jc + 1], e2_ps[:])
                w1c = vpool.tile([128, JC], f32, tag="w1c")
                nc.scalar.activation(w1c[:], e2c[:], AF.Exp, bias=0.0, scale=1.0)
                w4c = vpool.tile([128, JC], f32, tag="w4c")
                nc.scalar.activation(w4c[:], e2c[:], AF.Exp, bias=0.0, scale=0.25)
                w1_sb[v] = w1c
                w4_sb[v] = w4c
                for jc in range(JC):
                    t_ps = psC.tile([128, 128], fp16, tag="tp")
                    nc.tensor.transpose(
                        t_ps[:], hT[:, jc * 128:(jc + 1) * 128], ident16[:])
                    rt = rhspool.tile([128, H + 1], fp16, tag="rhs")
                    nc.scalar.copy(rt[:, 0:H], t_ps[:])
                    nc.vector.memset(rt[:, H:H + 1], 1.0)
                    rhs_sb[v][jc] = rt

            # single fused AllGather for all three views' r rows
            def ag_all():
                if SIM_NO_CC:
                    nc.sync.dma_start(
                        ago_all[:, :, :],
                        agi_all[:, :].partition_broadcast(NCORES))
                else:
                    nc.gpsimd.collective_compute(
                        "AllGather", OP.bypass,
                        replica_groups=[list(range(NCORES))],
                        ins=[agi_all.opt()], outs=[ago_all.opt()])

            # ---------- P3/P4 per view: attention + partials + RS ----------
            madj_all = {}

            def p3_adj(v):
                # AG-independent: prefetch+cast the adjacency tiles early
                for jc in range(JC):
                    madj = apool.tile([128, N], fp16, tag="madj")
                    nc.gpsimd.dma_start(
                        madj[:], adjm_d[v, jc * 128:(jc + 1) * 128, :])
                    madj_all[(v, jc)] = madj

            def p3(v):
                rb = e1pool.tile([128, N], fp16, tag="rb")
                nc.sync.dma_start(
                    rb[:].rearrange("p (k r) -> p k r", k=NCORES),
                    ago_all[:, v:v + 1, :].rearrange("k o r -> o k r")
                    .partition_broadcast(128))
                wms = []
                for jc in range(JC):
                    Wm = wmpool.tile([128, N], fp16, tag="Wm")
                    nc.vector.tensor_scalar(
                        out=Wm[:], in0=rb[:],
                        scalar1=w1_sb[v][:, jc:jc + 1],
                        scalar2=w4_sb[v][:, jc:jc + 1],
                        op0=OP.mult, op1=OP.max)
                    madj = madj_all.pop((v, jc))
                    nc.vector.tensor_tensor(Wm[:], Wm[:], madj[:], OP.mult)
                    
                    wms.append(Wm)
                # attention: PSUM banks packed 2 i-chunks wide; 4 i-chunks
                # staged per SBUF tile -> one 132KB partials DMA per core-slot
                # (on the scalar HWDGE ring, away from the big sync-ring loads)
                for ko in range(NCORES):
                    att4 = a4pool.tile([128, 4 * (H + 1)], fp16, tag="att4")
                    for pp in range(2):
                        att_ps = psA.tile([128, 2 * (H + 1)], f32, tag="att")
                        for c2 in range(2):
                            g = ko * 4 + pp * 2 + c2
                            osl = slice(c2 * (H + 1), (c2 + 1) * (H + 1))
                            for jc in range(JC):
                                nc.tensor.matmul(
                                    att_ps[:, osl],
                                    wms[jc][:, g * 128:(g + 1) * 128],
                                    rhs_sb[v][jc][:],
                                    start=(jc == 0), stop=(jc == JC - 1))
                        nc.scalar.copy(
                            att4[:, pp * 2 * (H + 1):(pp + 1) * 2 * (H + 1)],
                            att_ps[:])
                    nc.scalar.dma_start(
                        partials[v][ko, :, :]
                        .rearrange("(c p) h -> p c h", p=128),
                        att4[:].rearrange("p (c h) -> p c h", c=4))
                if SIM_NO_CC:
                    nc.sync.dma_start(rsouts[v][:, :], partials[v][0, :, :])
                else:
                    nc.gpsimd.collective_compute(
                        "ReduceScatter", OP.add,
                        replica_groups=[list(range(NCORES))],
                        ins=[partials[v].opt()], outs=[rsouts[v].opt()])

            # ---------- P5..P9 per view: fully per-chunk pipelined ----------
            mm_ps = psM.tile([C, NB], f32, tag="mm")

            mlpn_all = [None] * V

            def p5_mlp(v):
                # mlp branch: independent of the ReduceScatter -> compute
                # before the collectives to fill the startup dead zone
                mw16 = wpool.tile([128, DC * H], fp16, tag="mw16")
                nc.scalar.dma_start(mw16[:], mlpw16_d[v, :, :])
                rpm = wpool.tile([1, RP_X], fp16, tag="rpm")
                nc.scalar.dma_start(rpm[:], rp16_d[v, :, :])
                dt16 = dt16_all[v]
                mlpn = []
                for jc in range(JC):
                    mlp_ps = psB.tile([128, H], f32, tag="g")
                    for dc in range(DC):
                        nc.tensor.matmul(
                            mlp_ps[:],
                            dt16[:, dc * NB + jc * 128:dc * NB + (jc + 1) * 128],
                            mw16[:, dc * H:(dc + 1) * H],
                            start=(dc == 0), stop=False)
                    nc.tensor.matmul(mlp_ps[:], ones_row16[:],
                                     rpm[:, RP_MB:RP_MB + H],
                                     start=False, stop=True)
                    mn = mlppool.tile([128, H], fp16, tag="mlpn")
                    nc.scalar.copy(mn[:], mlp_ps[:])
                    mlpn.append(mn)
                mlpn_all[v] = mlpn

            def p5(v):
                gb_bc = wpool.tile([128, H], f32, tag="gb_bc")
                nc.sync.dma_start(gb_bc[:], gacb_d[v:v + 1, :].partition_broadcast(128))
                wg = wpool.tile([128, PG_X], fp16, tag="wg")
                nc.scalar.dma_start(wg[:], wp16_d[v, :, :])
                rp = wpool.tile([1, RP_X], fp16, tag="rp")
                nc.scalar.dma_start(rp[:], rp16_d[v, :, :])
                mlpn = mlpn_all[v]
                rsv4 = fe5pool.tile([128, JC * (H + 1)], fp16, tag="rsv4")
                nc.sync.dma_start(
                    rsv4[:],
                    rsouts[v][:, :].rearrange("(c p) h -> p c h", p=128))

                featT = fgpool.tile([128, NB], fp16, tag="fTall")
                for jc in range(JC):
                    nsl = slice(jc * 128, (jc + 1) * 128)
                    rsv = rsv4[:, jc * (H + 1):(jc + 1) * (H + 1)]
                    dcol = fe5pool.tile([128, 1], f32, tag="dcol")
                    nc.vector.tensor_copy(dcol[:], rsv[:, H:H + 1])
                    dinv = fe5pool.tile([128, 1], f32, tag="dinv")
                    nc.vector.reciprocal_approx_fast(out=dinv[:], in_=dcol[:])
                    featp = fe5pool.tile([128, H], f32, tag="featp")
                    nc.vector.scalar_tensor_tensor(
                        out=featp[:], in0=rsv[:, 0:H], scalar=dinv[:, 0:1],
                        in1=gb_bc[:], op0=OP.mult, op1=OP.add)
                    lk = fe5pool.tile([128, H], f32, tag="lk")
                    nc.scalar.activation(lk[:], featp[:], AF.Prelu,
                                         bias=0.0, scale=1.0, alpha=0.25)
                    feat = fe5pool.tile([128, H], fp16, tag="feat")
                    nc.vector.tensor_add(feat[:], lk[:], mlpn[jc][:])
                    t_ps = psC.tile([128, 128], fp16, tag="tp")
                    nc.tensor.transpose(t_ps[:], feat[:], ident16[:])
                    nc.scalar.copy(featT[:, nsl], t_ps[:])

                # batched K^T/V^T [o, n] for all 4 chunks
                kc_ps = psB.tile([128, NB], f32, tag="g")
                nc.tensor.matmul(kc_ps[:], wg[:, PG_KW:PG_KW + H], featT[:],
                                 start=True, stop=False)
                nc.tensor.matmul(kc_ps[:], rp[:, RP_KB:RP_KB + H],
                                 ones_nb16[:], start=False, stop=True)
                kb16 = fepool.tile([128, NB], fp16, tag="kb16")
                nc.scalar.copy(kb16[:], kc_ps[:])
                vc_ps = psB.tile([128, NB], f32, tag="g")
                nc.tensor.matmul(vc_ps[:], wg[:, PG_VW:PG_VW + H], featT[:],
                                 start=True, stop=False)
                nc.tensor.matmul(vc_ps[:], rp[:, RP_VB:RP_VB + H],
                                 ones_nb16[:], start=False, stop=True)
                vb16 = fepool.tile([128, NB], fp16, tag="vb16")
                nc.scalar.copy(vb16[:], vc_ps[:])

                # FE moment products, batched [o, NB] fp16
                kv = fepool.tile([128, NB], fp16, tag="kv")
                nc.vector.tensor_mul(kv[:], kb16[:], vb16[:])
                k2b = fepool.tile([128, NB], fp16, tag="k2b")
                nc.vector.tensor_mul(k2b[:], kb16[:], kb16[:])
                k2v = fepool.tile([128, NB], fp16, tag="k2v")
                nc.vector.tensor_mul(k2v[:], k2b[:], vb16[:])
                kpow = [None, kb16, k2b]
                kpv = {}
                for d in range(3, DFE + 1):
                    kd = fepool.tile([128, NB], fp16, tag=f"k{d}b")
                    nc.vector.tensor_mul(kd[:], kpow[d - 1][:], kb16[:])
                    kpow.append(kd)
                    kdv = fepool.tile([128, NB], fp16, tag=f"k{d}vb")
                    nc.vector.tensor_mul(kdv[:], kd[:], vb16[:])
                    kpv[d] = kdv
                mom_specs = [(0, vb16), (1, kv), (2, k2v)]
                for d in range(3, DFE + 1):
                    mom_specs.append((d, kpv[d]))
                mom_specs += [(DFE + 1, kb16), (DFE + 2, k2b)]
                for d in range(3, DFE + 1):
                    mom_specs.append((DFE + d, kpow[d]))

                aggT = fgpool.tile([128, NB], fp16, tag="aTall")
                for jc in range(JC):
                    nsl = slice(jc * 128, (jc + 1) * 128)
                    # Q [n, o] for this chunk
                    q_ps = psB.tile([128, H], f32, tag="g")
                    nc.tensor.matmul(q_ps[:], featT[:, nsl],
                                     wg[:, PG_QW:PG_QW + H],
                                     start=True, stop=False)
                    nc.tensor.matmul(q_ps[:], ones_row16[:],
                                     rp[:, RP_QB:RP_QB + H],
                                     start=False, stop=True)
                    qb_t = qpool.tile([128, H], fp16, tag="qb")
                    nc.vector.tensor_copy(qb_t[:], q_ps[:])

                    mt_ps = psB.tile([128, NMOM], f32, tag="g")
                    for col, prod in mom_specs:
                        nc.tensor.matmul(mt_ps[:, col:col + 1], prod[:, nsl],
                                         ones16c[:], start=True, stop=True)
                    mc = qpool.tile([128, NMOM], f32, tag="mcol")
                    nc.vector.tensor_mul(mc[:], mt_ps[:], fct_bc[:])

                    # FE assembly -> aggT chunk [o, n] fp16
                    pows = {1: qb_t}
                    for d in range(2, DFE + 1):
                        pd = spool.tile([128, H], fp16, tag=f"pow{d}")
                        nc.vector.tensor_mul(pd[:], pows[d - 1][:], qb_t[:])
                        pows[d] = pd
                    acc = None
                    for d in range(2, DFE + 1):
                        if acc is None:
                            acc = spool.tile([128, H], fp16, tag="accA")
                            nc.vector.tensor_scalar(
                                out=acc[:], in0=pows[d][:],
                                scalar1=mc[:, d:d + 1], scalar2=None, op0=OP.mult)
                        else:
                            nxt = spool.tile([128, H], fp16,
                                             tag="accB" if d % 2 else "accA")
                            nc.vector.scalar_tensor_tensor(
                                out=nxt[:], in0=pows[d][:], scalar=mc[:, d:d + 1],
                                in1=acc[:], op0=OP.mult, op1=OP.add)
                            acc = nxt
                    dacc = None
                    for d in range(2, DFE + 1):
                        nrow = DFE + d
                        if dacc is None:
                            dacc = spool.tile([128, H], fp16, tag="daccA")
                            nc.vector.tensor_scalar(
                                out=dacc[:], in0=pows[d][:],
                                scalar1=mc[:, nrow:nrow + 1], scalar2=None,
                                op0=OP.mult)
                        else:
                            nxt = spool.tile([128, H], fp16,
                                             tag="daccB" if d % 2 else "daccA")
                            nc.vector.scalar_tensor_tensor(
                                out=nxt[:], in0=pows[d][:],
                                scalar=mc[:, nrow:nrow + 1],
                                in1=dacc[:], op0=OP.mult, op1=OP.add)
                            dacc = nxt
                    denh = spool.tile([128, H], f32, tag="hpart")
                    nc.vector.tensor_scalar(out=denh[:], in0=qb_t[:],
                                            scalar1=mc[:, DFE + 1:DFE + 2],
                                            scalar2=float(H),
                                            op0=OP.mult, op1=OP.add)
                    den = spool.tile([128, H], f32, tag="nd")
                    nc.vector.tensor_add(den[:], denh[:], dacc[:])
                    rden = spool.tile([128, H], f32, tag="rden")
                    nc.vector.reciprocal_approx_fast(out=rden[:], in_=den[:])
                    numh = spool.tile([128, H], f32, tag="hpart")
                    nc.vector.tensor_scalar(out=numh[:], in0=qb_t[:],
                                            scalar1=mc[:, 1:2], scalar2=mc[:, 0:1],
                                            op0=OP.mult, op1=OP.add)
                    num = spool.tile([128, H], f32, tag="nd")
                    nc.vector.tensor_add(num[:], numh[:], acc[:])
                    agg = spool.tile([128, H], fp16, tag="agg")
                    nc.vector.tensor_mul(agg[:], num[:], rden[:])
                    at_ps = psC.tile([128, 128], fp16, tag="tp")
                    nc.tensor.transpose(at_ps[:], agg[:], ident16[:])
                    nc.scalar.copy(aggT[:, nsl], at_ps[:])

                # batched fc + relu; conf gate; classifier accum
                f2_ps = psB.tile([128, NB], f32, tag="g")
                nc.tensor.matmul(f2_ps[:], wg[:, PG_FW:PG_FW + H], aggT[:],
                                 start=True, stop=False)
                nc.tensor.matmul(f2_ps[:], rp[:, RP_FB:RP_FB + H],
                                 ones_nb16[:], start=False, stop=True)
                f2c = fgpool.tile([128, NB], fp16, tag="f2c")
                nc.scalar.activation(f2c[:], f2_ps[:], AF.Relu)
                cf_ps = psB.tile([1, NB], f32, tag="g")
                nc.tensor.matmul(cf_ps[:], wg[:, PG_CW:PG_CW + 1],
                                 f2c[:], start=True, stop=False)
                nc.tensor.matmul(cf_ps[:], rp[:, RP_CB:RP_CB + 1],
                                 ones_nb16[:], start=False, stop=True)
                cf_row = fepool.tile([1, NB], fp16, tag="cf_row")
                nc.scalar.copy(cf_row[:], cf_ps[:])
                cb_ps = psB.tile([128, NB], f32, tag="g")
                nc.tensor.matmul(cb_ps[:], ones_row16[:], cf_row[:],
                                 start=True, stop=True)
                gtf = fgpool.tile([128, NB], fp16, tag="gated")
                nc.vector.tensor_mul(gtf[:], f2c[:], cb_ps[:])
                nc.tensor.matmul(mm_ps[:], wg[:, PG_MMW:PG_MMW + C],
                                 gtf[:], start=(v == 0), stop=False)

            # ---------- schedule: software-pipeline the three views ----------
            warmup_cc()
            p1(0)
            p1(1)
            p1(2)
            ag_all()
            p3_adj(0)
            p3_adj(1)
            p5_mlp(0)
            p5_mlp(1)
            p5_mlp(2)
            p3(0)
            p3_adj(2)
            p3(1)
            p3(2)
            p5(0)
            p5(1)
            p5(2)

            # ---------- P9: bias + output ----------
            rp2 = wpool.tile([1, RP_X], fp16, tag="rp")
            nc.scalar.dma_start(rp2[:], rp16_d[V - 1, :, :])
            nc.tensor.matmul(mm_ps[:], rp2[:, RP_MMB:RP_MMB + C],
                             ones_nb16[:], start=False, stop=True)
            lg = fepool.tile([C, NB], fp16, tag="lg")
            nc.scalar.copy(lg[:], mm_ps[:])
            for jc in range(JC):
                lt_ps = psB.tile([128, C], fp16, tag="g")
                nc.tensor.transpose(lt_ps[:], lg[:, jc * 128:(jc + 1) * 128],
                                    ident16[0:C, 0:C])
                osb = fe5pool.tile([128, C], f32, tag="osb")
                nc.vector.tensor_copy(osb[:], lt_ps[:])
                nc.sync.dma_start(out_d[jc * 128:(jc + 1) * 128, :], osb[:])
    return nc


def _prep_inputs(inputs):
    adj = np.asarray(inputs["adj"])
    s = math.sqrt(H)
    adjT = np.ascontiguousarray(adj.transpose(0, 2, 1)).astype(np.float32)
    if ACCUM_ADD:
        adjT = (adjT - 1.0) * MASK_BIG
    adjT = adjT.astype(ml_dtypes.float8_e5m2)
    data = np.asarray(inputs["data"], dtype=np.float32)
    dataT = data.transpose(0, 2, 1)  # [V, D, N]
    f = np.float32
    wpack = np.zeros((V, 128, PK_X), np.float16)
    wpack[:, :, PK_A1] = np.asarray(inputs["a1"], f)
    wpack[:, :, PK_A2] = np.asarray(inputs["a2"], f)
    wp16 = np.zeros((V, 128, PG_X), np.float16)
    wp16[:, :, PG_QW:PG_QW + H] = np.asarray(inputs["q_w"], f) / s
    wp16[:, :, PG_KW:PG_KW + H] = np.asarray(inputs["k_w"], f)
    wp16[:, :, PG_VW:PG_VW + H] = np.asarray(inputs["v_w"], f)
    wp16[:, :, PG_FW:PG_FW + H] = np.asarray(inputs["fc_w"], f)
    wp16[:, :, PG_CW] = np.asarray(inputs["conf_w"], f).reshape(V, H)
    wp16[:, :, PG_MMW:PG_MMW + C] = np.asarray(inputs["mm_w"], f).reshape(V, H, C)
    rp16 = np.zeros((V, 1, RP_X), np.float16)
    rp16[:, 0, RP_QB:RP_QB + H] = np.asarray(inputs["q_b"], f) / s
    rp16[:, 0, RP_KB:RP_KB + H] = np.asarray(inputs["k_b"], f)
    rp16[:, 0, RP_VB:RP_VB + H] = np.asarray(inputs["v_b"], f)
    rp16[:, 0, RP_MB:RP_MB + H] = np.asarray(inputs["mlp_b"], f)
    rp16[:, 0, RP_FB:RP_FB + H] = np.asarray(inputs["fc_b"], f)
    rp16[:, 0, RP_CB] = np.asarray(inputs["conf_b"], f).reshape(V)
    rp16[:, 0, RP_MMB:RP_MMB + C] = np.asarray(inputs["mm_b"], f)[None, :]

    def chunk4(x, inner):  # [V, DC*128, inner] -> [V, 128, DC*inner]
        return np.ascontiguousarray(
            x.reshape(V, DC, 128, inner).transpose(0, 2, 1, 3)
             .reshape(V, 128, DC * inner))

    common = {
        "gacw16": chunk4(np.asarray(inputs["gac_w"], f), H).astype(np.float16),
        "mlpw16": chunk4(np.asarray(inputs["mlp_w"], f), H).astype(np.float16),
        "gacb": np.asarray(inputs["gac_b"], f),
        "wpack": wpack, "wp16": wp16, "rp16": rp16,
    }
    in_maps = []
    for c in range(NCORES):
        r0, r1 = c * NB, (c + 1) * NB
        m = dict(common)
        m["adjm"] = np.ascontiguousarray(adjT[:, r0:r1, :])
        dslice = np.ascontiguousarray(dataT[:, :, r0:r1])  # [V, D, NB]
        m["dataT16"] = chunk4(dslice, NB).astype(np.float16)
        in_maps.append(m)
    return in_maps


def kernel(**inputs):
    if "nc" not in _CACHE:
        nc = build_nc()
        nc.compile()
        _CACHE["nc"] = nc
    nc = _CACHE["nc"]
    in_maps = _prep_inputs(inputs)
    res = run_bass_kernel_spmd(nc, in_maps, list(range(NCORES)))
    out = np.concatenate([res.results[c]["out"] for c in range(NCORES)], axis=0)
    return out.astype(np.float32)


if __name__ == "__main__":
    nc = build_nc()
    print("build ok; instructions:",
          sum(len(b.instructions) for f in nc.m.functions for b in f.blocks))
    nc.compile()
    print("bacc compile ok")
